# revision 39
# baseline (speedup 1.0000x reference)
"""CGCoupler Trainium2 Bass kernel.

out[n, ro[k]] += x1[n, r1[k]] * x2[n, r2[k]] * cg[k]  for all k, rows n.

Because the CG index tables address contiguous channel runs, the whole op
decomposes into ~147 contiguous-slice FMAs per row:
    out[:, o:o+d] += c * x1[:, a:a+d] * x2[:, b:b+d]
with d in {32, 64}.  Rows live on the 128 SBUF partitions, the 640-wide
feature dim on the free axis, T=4 row-tiles folded per instruction.

All device data is fp16 (inputs cast on host, output cast back): halves
HBM traffic and enables the DVE 2x perf mode for the TensorTensor
products.  The accumulation work is spread across engines:

  * DVE computes the shared products P = x1*x2 (70 unique slice pairs).
  * PE (tensor engine) does most of the scatter-add: matmul with a
    diag(c) stationary is a scaled copy-accumulate into PSUM, so each
    CG term is one matmul; PSUM accumulation is free and exact (fp32).
    Output runs are packed into PSUM banks (one 2KB bank = 128 output
    positions x T row-tiles); the first matmul per bank sets
    start_tensor_calc (hardware zeros the bank), the last sets stop.
  * Act (scalar engine) evicts PSUM -> fp16 output tile.
  * Pool (gpsimd) takes the leftover components as scalar_tensor_tensor
    accumulations directly in SBUF.

Data-parallel across 8 NeuronCores: each core processes 2048 rows.
"""
import numpy as np

N_CORES = 8
P_DIM = 128
T_FOLD = 4            # row-tiles folded per instruction group
BANK_F32 = 512        # PSUM bank capacity in fp32 elements
BANK_SLOTS = BANK_F32 // T_FOLD   # output positions per PSUM bank
N_PE_BANKS = 4        # banks per PSUM buffer (x2 buffers = all 8)

# cost-model ns-per-(row-elem) rates (x16 free-elems per core) for balancing
RATE_PE = 6.7
RATE_POOL_STT = 1e9
RATE_DVE_STT = 17.3
RATE_DVE_TT = 10.8
RATE_POOL_TT = 35.5
PE_CAP = 512          # PSUM slots available to PE-assigned output positions

_BUILD_CACHE = {}


# ----------------------------------------------------------------------------
# Planning
# ----------------------------------------------------------------------------

def _extract_sliceops(cg, r1, r2, ro):
    M = len(cg)
    ops = []
    k = 0
    while k < M:
        j = k + 1
        while (j < M and r1[j] == r1[j-1] + 1 and r2[j] == r2[j-1] + 1
               and ro[j] == ro[j-1] + 1 and cg[j] == cg[k]):
            j += 1
        ops.append((int(r1[k]), int(r2[k]), int(ro[k]), j - k, float(cg[k])))
        k = j
    return ops


def _components(ops):
    """Union output runs (o, d) into overlap-connected components."""
    runs = sorted(set((o, d) for (_, _, o, d, _) in ops))
    parent = {r: r for r in runs}

    def find(r):
        while parent[r] != r:
            parent[r] = parent[parent[r]]
            r = parent[r]
        return r

    for i, ri in enumerate(runs):
        for rj in runs[i+1:]:
            if ri[0] < rj[0] + rj[1] and rj[0] < ri[0] + ri[1]:
                ra, rb = find(ri), find(rj)
                if ra != rb:
                    parent[ra] = rb
    comp_of = {r: find(r) for r in runs}
    # canonical comp descriptor: (lo, hi)
    spans = {}
    for r in runs:
        c = comp_of[r]
        lo, hi = spans.get(c, (10**9, -1))
        spans[c] = (min(lo, r[0]), max(hi, r[0] + r[1]))
    return {r: spans[comp_of[r]] for r in runs}


def _build_plan(cg, r1, r2, ro, out_dim, pe_cap=PE_CAP):
    ops = _extract_sliceops(cg, r1, r2, ro)

    # ---- shared products -------------------------------------------------
    pair_order, pair_idx = [], {}
    for (a, b, o, d, c) in ops:
        key = (a, b, d)
        if key not in pair_idx:
            pair_idx[key] = len(pair_order)
            pair_order.append(key)
    slot, cur = {}, 0
    for key in pair_order:
        slot[key] = cur
        cur += key[2]
    psize = cur

    prod_instrs = []
    i = 0
    while i < len(pair_order):
        a0, b0, d0 = pair_order[i]
        s0 = slot[pair_order[i]]
        j = i + 1
        da = db = ds = None
        while j < len(pair_order):
            a1, b1, d1 = pair_order[j]
            if d1 != d0:
                break
            nda = a1 - pair_order[j-1][0]
            ndb = b1 - pair_order[j-1][1]
            nds = slot[pair_order[j]] - slot[pair_order[j-1]]
            if da is None:
                da, db, ds = nda, ndb, nds
            elif (nda, ndb, nds) != (da, db, ds):
                break
            j += 1
        n = j - i
        if n == 1:
            da = db = ds = 0
        prod_instrs.append(dict(pslot=s0, a=a0, b=b0, d=d0,
                                da=da, db=db, ds=ds, n=n))
        i = j

    # ---- accumulation ops with component labels --------------------------
    comp_of = _components(ops)
    accs = [dict(o=o, pslot=slot[(a, b, d)], c=c, d=d, comp=comp_of[(o, d)])
            for (a, b, o, d, c) in ops]

    comps = sorted(set(q['comp'] for q in accs))
    comp_elems = {cm: 0 for cm in comps}
    comp_nops = {cm: 0 for cm in comps}
    for q in accs:
        comp_elems[q['comp']] += q['d']
        comp_nops[q['comp']] += 1

    # ---- engine assignment ----------------------------------------------
    # PE takes the densest components (accum elems per output position)
    # until the PSUM slot budget is filled; leftovers balance DVE/Pool.
    prod_elems_total = sum(p['n'] * p['d'] for p in prod_instrs)
    pe_width = 0
    comp_eng = {}
    for cm in sorted(comps, key=lambda cm: -comp_elems[cm] / (cm[1] - cm[0])):
        w = cm[1] - cm[0]
        if pe_width + w <= pe_cap:
            comp_eng[cm] = 'pe'
            pe_width += w
    busy = {'dve': 1500.0 + prod_elems_total * RATE_DVE_TT, 'pool': 0.0}
    for cm in sorted(comps, key=lambda cm: -comp_elems[cm]):
        if cm in comp_eng:
            continue
        elems = comp_elems[cm]
        td = busy['dve'] + elems * RATE_DVE_STT
        tp = busy['pool'] + elems * RATE_POOL_STT
        if td <= tp:
            comp_eng[cm] = 'dve'
            busy['dve'] = td
        else:
            comp_eng[cm] = 'pool'
            busy['pool'] = tp

    # products all on DVE: a second producer engine would add a semaphore
    # wait to every consumer's first instruction (walrus wait-slot limits)
    for pi in prod_instrs:
        pi['eng'] = 'dve'

    # ---- PSUM bank packing (span order, first-fit into 128-slot banks) ---
    n_banks = (pe_cap + BANK_SLOTS - 1) // BANK_SLOTS
    pe_comps = [cm for cm in comps if comp_eng[cm] == 'pe']
    bank_of, base_of = {}, {}
    bank_fill = [0] * n_banks
    for cm in pe_comps:                      # span order keeps evicts merged
        w = cm[1] - cm[0]
        for b in range(n_banks):
            if bank_fill[b] + w <= BANK_SLOTS:
                bank_of[cm], base_of[cm] = b, bank_fill[b]
                bank_fill[b] += w
                break
        else:
            raise RuntimeError("PSUM bank packing failed")

    # eviction instructions: maximal runs of PE comps, contiguous in both
    # psum slots and output positions
    evicts = []
    for cm in pe_comps:
        b, s, o, w = bank_of[cm], base_of[cm], cm[0], cm[1] - cm[0]
        if (evicts and evicts[-1]['bank'] == b
                and evicts[-1]['slot'] + evicts[-1]['w'] == s
                and evicts[-1]['o'] + evicts[-1]['w'] == o):
            evicts[-1]['w'] += w
        else:
            evicts.append(dict(bank=b, slot=s, o=o, w=w))

    # ---- per-engine accumulation instructions ----------------------------
    # PE: psum slot per op; merge same-(c,bank) contiguous ops, cap w<=128
    pe_ops = []
    for q in accs:
        cm = q['comp']
        if comp_eng[cm] != 'pe':
            continue
        pe_ops.append(dict(bank=bank_of[cm], slot=base_of[cm] + q['o'] - cm[0],
                           pslot=q['pslot'], c=q['c'], d=q['d'], o=q['o']))
    # PSUM cells must be uniformly first-write or all-accumulate within one
    # matmul: split ops into a first-touch phase (covers every cell once,
    # wide-first) and an accumulate phase; c-grouped within each phase.
    covered = np.zeros(out_dim, bool)
    for q in sorted(pe_ops, key=lambda q: (-q['d'], q['c'], q['o'])):
        rng = slice(q['o'], q['o'] + q['d'])
        if not covered[rng].any():
            q['ft'] = True
        else:
            assert covered[rng].all(), "partial first-touch in PE domain"
            q['ft'] = False
        covered[rng] = True
    cmin = {}
    for q in pe_ops:
        cmin[q['c']] = min(cmin.get(q['c'], 1 << 30), q['pslot'])
    pe_ops.sort(key=lambda q: (not q['ft'], cmin[q['c']], q['c'], q['pslot'],
                               q['bank'], q['slot']))
    pe_instrs = []
    i = 0
    while i < len(pe_ops):
        q0 = pe_ops[i]
        j = i + 1
        while j < len(pe_ops):
            q1, qp = pe_ops[j], pe_ops[j-1]
            if (q1['c'] != q0['c'] or q1['bank'] != q0['bank']
                    or q1['d'] != q0['d'] or q1['ft'] != q0['ft']
                    or q1['slot'] - qp['slot'] != q0['d']
                    or q1['pslot'] - qp['pslot'] != q0['d']
                    or (j - i + 1) * q0['d'] > BANK_SLOTS):
                break
            j += 1
        n = j - i
        pe_instrs.append(dict(bank=q0['bank'], slot=q0['slot'], pslot=q0['pslot'],
                              c=q0['c'], w=n * q0['d']))
        i = j
    # stop flags: last instruction per bank in emission order.  Banks are
    # OPENED by a sacrificial zero-add matmul (start_tensor_calc zeros the
    # bank and absorbs the PSUM write-after-write wait), so no real
    # instruction carries start.
    last_idx = {}
    for idx, q in enumerate(pe_instrs):
        q['start'] = False
        last_idx[q['bank']] = idx
        q['stop'] = False
    for b, idx in last_idx.items():
        pe_instrs[idx]['stop'] = True
    bank_stop = {b: (pe_instrs[i]['slot'], pe_instrs[i]['w'])
                 for b, i in last_idx.items()}

    # DVE/Pool: TS first-touch then STT, merged contiguous same-c runs
    sb_instrs = []
    for eng in ('dve', 'pool'):
        qs = [q for q in accs if comp_eng[q['comp']] == eng]
        covered = np.zeros(out_dim, bool)
        qs.sort(key=lambda q: (-q['d'], q['c'], q['o'], q['pslot']))
        for q in qs:
            rng = slice(q['o'], q['o'] + q['d'])
            if not covered[rng].any():
                q['kind'] = 'TS'
            else:
                assert covered[rng].all(), "partial first-touch"
                q['kind'] = 'STT'
            covered[rng] = True
        qs.sort(key=lambda q: (q['kind'] != 'TS', -q['d'], q['c'], q['o'],
                               q['pslot']))
        i = 0
        while i < len(qs):
            q0 = qs[i]
            j = i + 1
            while j < len(qs):
                q1, qp = qs[j], qs[j-1]
                if (q1['kind'] != q0['kind'] or q1['d'] != q0['d']
                        or q1['c'] != q0['c']
                        or q1['o'] - qp['o'] != q0['d']
                        or q1['pslot'] - qp['pslot'] != q0['d']):
                    break
                j += 1
            n = j - i
            sb_instrs.append(dict(kind=q0['kind'], o=q0['o'], pslot=q0['pslot'],
                                  c=q0['c'], d=q0['d'], n=n, eng=eng))
            i = j

    cvals = sorted(set(q['c'] for q in pe_instrs))
    last_pi = prod_instrs[-1]
    return dict(psize=psize, prod_instrs=prod_instrs, pe_instrs=pe_instrs,
                sb_instrs=sb_instrs, evicts=evicts, cvals=cvals,
                n_banks=n_banks, xl1=last_pi['a'], xl2=last_pi['b'],
                bank_stop=bank_stop)


# ----------------------------------------------------------------------------
# Bass program
# ----------------------------------------------------------------------------

def _build_bass(plan, rows_per_core, rep_dim, out_dim, repeat=1):
    import concourse.bass as bass
    import concourse.mybir as mybir
    from concourse.ap import AP
    from concourse.tile import TileContext
    import concourse.tile as _tile_mod
    from concourse.vector_clock import ScopedClock as _ScopedClock

    # The kernel-tail Drain instruction waits on every proc lane with
    # outstanding ticks, but its CTRL ISA struct only has room for a couple
    # of embedded sync-wait commands ("Too many sync wait commands" in
    # walrus codegen otherwise).  Split the global-clock wait across
    # several Drain instructions (waits already observed by the SP engine
    # are elided by add_sem_waits).
    if not getattr(_tile_mod.TileContext, '_cg_drain_patched', False):
        def _split_drain_and_barrier(self, tick_clock, wait_clock):
            gc = tick_clock.global_clock
            VC = type(gc)
            procs = []
            for p in range(27):
                t = gc.peek_next(p) - 1
                if t > 0:
                    procs.append((p, t))
            for i in range(0, len(procs), 1):
                pc = VC()
                for p, t in procs[i:i + 1]:
                    for _ in range(t):
                        pc.advance(p)
                d = self.nc.sync.drain()
                wait_clock.add_sem_waits(d.ins, _ScopedClock({None: pc}))
            self.nc.all_engine_barrier()
            popped = self.nc._tile_sem_poison_stack.pop()
            assert popped is self._sem_poison
            self.nc.clear_and_free_semaphores(list(self.sems.allocated().values()))
            self.nc.all_engine_barrier()

        _tile_mod.TileContext._drain_and_barrier = _split_drain_and_barrier
        _tile_mod.TileContext._cg_drain_patched = True

    f16 = mybir.dt.float16
    f32 = mybir.dt.float32
    T = T_FOLD
    n_groups = rows_per_core // (P_DIM * T)
    assert rows_per_core == n_groups * P_DIM * T

    nc = bass.Bass("TRN2")
    x1d = nc.declare_dram_parameter("x1", [rows_per_core, rep_dim], f16, isOutput=False)
    x2d = nc.declare_dram_parameter("x2", [rows_per_core, rep_dim], f16, isOutput=False)
    outd = nc.declare_dram_parameter("out", [rows_per_core, out_dim], f16, isOutput=True)

    def ap_custom(tile, base, dims):
        a = tile[:]
        aplist = [list(a.ap[0])] + [[s, n] for (s, n) in dims]
        return AP(a.tensor, a.offset + base, aplist)

    cvals = plan['cvals']
    with TileContext(nc) as tc:
        with (
            tc.tile_pool(name="const", bufs=1) as cstp,
            tc.tile_pool(name="io", bufs=4) as iop,
            tc.tile_pool(name="pp", bufs=4) as ppp,
            tc.tile_pool(name="ps", bufs=2, space="PSUM") as psp,
        ):
            # one-time: diag(c) stationary tiles built from an identity
            IDT = cstp.tile([P_DIM, P_DIM], f16, tag="IDT")
            nc.gpsimd.memset(IDT[:], 1.0)
            nc.gpsimd.affine_select(
                IDT[:], IDT[:], pattern=[[1, P_DIM]],
                compare_op=mybir.AluOpType.is_equal, fill=0.0,
                base=0, channel_multiplier=-1)
            DIAG = {}
            for c in cvals:
                D = cstp.tile([P_DIM, P_DIM], f16, tag=f"D{len(DIAG)}")
                nc.scalar.mul(D[:], IDT[:], float(c))
                DIAG[c] = D
            ZT = cstp.tile([P_DIM, BANK_F32], f16, tag="ZT")
            nc.gpsimd.memset(ZT[:], 0.0)

            def dram_group_ap(dram, g, width):
                # [128p, T, width] view of rows [g*T*128, (g+1)*T*128)
                a = dram[:]
                return AP(a.tensor, g * T * P_DIM * width,
                          [[width, P_DIM], [P_DIM * width, T], [1, width]])

            csz = plan['psize']
            pool_qs = [q for q in plan['sb_instrs'] if q['eng'] == 'pool']
            P_prev = P_prev2 = None
            for it in range(n_groups * repeat):
                g = it % n_groups
                warm = it >= 1   # a previous iteration exists
                X1 = iop.tile([P_DIM, T * rep_dim], f16, tag="X1")
                X2 = iop.tile([P_DIM, T * rep_dim], f16, tag="X2")
                O = iop.tile([P_DIM, T * out_dim], f16, tag="O")
                PS = psp.tile([P_DIM, plan['n_banks'] * BANK_F32], f32, tag="PS")
                SCR = iop.tile([P_DIM, 16], f16, tag="SCR")
                P = ppp.tile([P_DIM, T * csz], f16, tag="P")

                # Every ISA struct holds only 1-2 embedded sync waits, and
                # the tile framework expands transitive vector clocks into
                # explicit waits.  Each engine therefore "observes the
                # world" through a chain of absorber instructions, each
                # adding at most ~2 new clock components, so the hot
                # instructions carry at most one wait.
                nc.gpsimd.dma_start(X1[:], dram_group_ap(x1d, g, rep_dim))
                nc.gpsimd.dma_start(X2[:], dram_group_ap(x2d, g, rep_dim))
                # DVE absorbers: DMA completions, then Pool's marker
                nc.vector.tensor_copy(SCR[:, 0:2], X1[:, 0:2])
                nc.vector.tensor_copy(SCR[:, 2:4], X2[:, 0:2])
                # Act absorber: observes DVE (and the DMA sem transitively)
                nc.scalar.copy(SCR[:, 12:14], SCR[:, 0:2])

                for pi in plan['prod_instrs']:
                    dims = [(csz, T), (pi['ds'], pi['n']), (1, pi['d'])]
                    eng = nc.vector if pi['eng'] == 'dve' else nc.gpsimd
                    eng.tensor_tensor(
                        ap_custom(P, pi['pslot'], dims),
                        ap_custom(X1, pi['a'],
                                  [(rep_dim, T), (pi['da'], pi['n']), (1, pi['d'])]),
                        ap_custom(X2, pi['b'],
                                  [(rep_dim, T), (pi['db'], pi['n']), (1, pi['d'])]),
                        mybir.AluOpType.mult,
                    )

                # PE absorbers: garbage ldweights reading the observer
                # scratch; each pulls in the producer's clock so the real
                # matmuls carry at most one wait.  The first real ldweights
                # overwrites the PE array anyway.
                nc.tensor.ldweights(SCR[:, 0:2])
                nc.tensor.ldweights(SCR[:, 12:14])

                # PE scatter-add: diag(c) matmuls accumulating in PSUM.
                # One matmul per row-tile t: a single matmul's PSUM out AP
                # must be flat and stay within one 2KB bank.
                c0 = cvals[0]
                for b in range(plan['n_banks']):
                    out_ap = ap_custom(PS, b * BANK_F32, [(1, BANK_F32)])
                    nc.tensor.matmul(out_ap, DIAG[c0][:], ZT[:],
                                     start=True, stop=False)
                for qi in plan['pe_instrs']:
                    for t in range(T):
                        out_ap = ap_custom(
                            PS, qi['bank'] * BANK_F32 + t * BANK_SLOTS + qi['slot'],
                            [(1, qi['w'])])
                        mov_ap = ap_custom(P, t * csz + qi['pslot'],
                                           [(1, qi['w'])])
                        nc.tensor.matmul(out_ap, DIAG[qi['c']][:], mov_ap,
                                         start=False,
                                         stop=qi['stop'] and t == T - 1)

                # DVE/Pool: leftover components in SBUF.  Per-engine
                # absorbers: read the LAST product's range (observes the max
                # DVE tick -> later product-RAW waits elide) and touch O to
                # absorb the WAR against the out-DMA two groups back.
                first_o = {}
                for qi in plan['sb_instrs']:
                    first_o.setdefault(qi['eng'], qi['o'])
                for k, (eng_name, o0) in enumerate(sorted(first_o.items())):
                    eng = nc.vector if eng_name == 'dve' else nc.gpsimd
                    eng.tensor_copy(SCR[:, 6 + 2*k:8 + 2*k],
                                    P[:, T * csz - 2:T * csz])
                    eng.tensor_copy(ap_custom(O, o0, [(1, 2)]),
                                    SCR[:, 6 + 2*k:8 + 2*k])
                for qi in plan['sb_instrs']:
                    w = qi['n'] * qi['d']
                    o_ap = ap_custom(O, qi['o'], [(out_dim, T), (1, w)])
                    p_ap = ap_custom(P, qi['pslot'], [(csz, T), (1, w)])
                    eng = nc.vector if qi['eng'] == 'dve' else nc.gpsimd
                    if qi['kind'] == 'TS':
                        eng.tensor_scalar_mul(o_ap, p_ap, float(qi['c']))
                    else:
                        eng.scalar_tensor_tensor(
                            out=o_ap, in0=p_ap, scalar=float(qi['c']),
                            in1=o_ap,
                            op0=mybir.AluOpType.mult,
                            op1=mybir.AluOpType.add,
                        )

                # Act: absorb the out-DMA(g-2) WAR once, then evict
                # PSUM -> fp16 output tile (single PE wait each)
                if warm and plan['evicts']:
                    ev0 = plan['evicts'][0]['o']
                    nc.scalar.copy(ap_custom(O, ev0, [(1, 2)]), SCR[:, 12:14])
                for ev in plan['evicts']:
                    src = ap_custom(PS, ev['bank'] * BANK_F32 + ev['slot'],
                                    [(BANK_SLOTS, T), (1, ev['w'])])
                    dst = ap_custom(O, ev['o'], [(out_dim, T), (1, ev['w'])])
                    nc.scalar.copy(dst, src)
                # Act gates the non-Act writers of O (DVE accums) with an
                # in-place 2-element copy carrying that single wait, then
                # issues the out-DMA itself: all O writers are then its own
                # in-order engine, leaving only the DMA queue wait.
                dve_qs = [q for q in plan['sb_instrs'] if q['eng'] == 'dve']
                if dve_qs:
                    ql = dve_qs[-1]
                    ap = ap_custom(O, ql['o'] + ql['n'] * ql['d'] - 2, [(1, 2)])
                    nc.scalar.copy(ap, ap)
                nc.scalar.dma_start(dram_group_ap(outd, g, out_dim), O[:])
                P_prev2 = P_prev
                P_prev = P
    return nc


# ----------------------------------------------------------------------------
# Entry point
# ----------------------------------------------------------------------------

def kernel(x1, x2, cg_tilde, repids_in1, repids_in2, repids_out, out_dim):
    from concourse.bass_utils import run_bass_kernel_spmd

    x1 = np.asarray(x1, dtype=np.float16)
    x2 = np.asarray(x2, dtype=np.float16)
    cg = np.asarray(cg_tilde, dtype=np.float32)
    r1 = np.asarray(repids_in1).astype(np.int64)
    r2 = np.asarray(repids_in2).astype(np.int64)
    ro = np.asarray(repids_out).astype(np.int64)
    out_dim = int(out_dim)

    n, rep_dim = x1.shape
    rows_per_core = n // N_CORES

    key = (rows_per_core, rep_dim, out_dim, cg.tobytes(), r1.tobytes(),
           r2.tobytes(), ro.tobytes())
    cache_key = hash(key)
    if cache_key not in _BUILD_CACHE:
        plan = _build_plan(cg, r1, r2, ro, out_dim)
        nc = _build_bass(plan, rows_per_core, rep_dim, out_dim)
        _BUILD_CACHE[cache_key] = nc
    nc = _BUILD_CACHE[cache_key]

    in_maps = [
        {"x1": x1[i*rows_per_core:(i+1)*rows_per_core],
         "x2": x2[i*rows_per_core:(i+1)*rows_per_core]}
        for i in range(N_CORES)
    ]
    res = run_bass_kernel_spmd(nc, in_maps, list(range(N_CORES)))
    out = np.concatenate([res.results[i]["out"] for i in range(N_CORES)], axis=0)
    return out.astype(np.float32)


# revision 48
# speedup vs baseline: 3.8761x; 3.8761x over previous
"""CGCoupler Trainium2 Bass kernel.

out[n, ro[k]] += x1[n, r1[k]] * x2[n, r2[k]] * cg[k]  for all k, rows n.

Because the CG index tables address contiguous channel runs, the whole op
decomposes into ~147 contiguous-slice FMAs per row:
    out[:, o:o+d] += c * x1[:, a:a+d] * x2[:, b:b+d]
with d in {32, 64}.  Rows live on the 128 SBUF partitions, the 640-wide
feature dim on the free axis, T=4 row-tiles folded per instruction.

All device data is fp16 (inputs cast on host, output cast back): halves
HBM traffic and enables the DVE 2x perf mode for the TensorTensor
products.  The accumulation work is spread across engines:

  * DVE computes the shared products P = x1*x2 (70 unique slice pairs).
  * PE (tensor engine) does most of the scatter-add: matmul with a
    diag(c) stationary is a scaled copy-accumulate into PSUM, so each
    CG term is one matmul; PSUM accumulation is free and exact (fp32).
    Output runs are packed into PSUM banks (one 2KB bank = 128 output
    positions x T row-tiles); the first matmul per bank sets
    start_tensor_calc (hardware zeros the bank), the last sets stop.
  * Act (scalar engine) evicts PSUM -> fp16 output tile.
  * Pool (gpsimd) takes the leftover components as scalar_tensor_tensor
    accumulations directly in SBUF.

Data-parallel across 8 NeuronCores: each core processes 2048 rows.
"""
import numpy as np

N_CORES = 8
P_DIM = 128
T_FOLD = 4            # row-tiles folded per instruction group
BANK_F32 = 512        # PSUM bank capacity in fp32 elements
BANK_SLOTS = BANK_F32 // T_FOLD   # output positions per PSUM bank
N_PE_BANKS = 4        # banks per PSUM buffer (x2 buffers = all 8)

# cost-model ns-per-(row-elem) rates (x16 free-elems per core) for balancing
RATE_PE = 6.7
RATE_POOL_STT = 1e9
RATE_DVE_STT = 17.3
RATE_DVE_TT = 10.8
RATE_POOL_TT = 35.5
PE_CAP = 512          # PSUM slots available to PE-assigned output positions

_BUILD_CACHE = {}


# ----------------------------------------------------------------------------
# Planning
# ----------------------------------------------------------------------------

def _extract_sliceops(cg, r1, r2, ro):
    M = len(cg)
    ops = []
    k = 0
    while k < M:
        j = k + 1
        while (j < M and r1[j] == r1[j-1] + 1 and r2[j] == r2[j-1] + 1
               and ro[j] == ro[j-1] + 1 and cg[j] == cg[k]):
            j += 1
        ops.append((int(r1[k]), int(r2[k]), int(ro[k]), j - k, float(cg[k])))
        k = j
    return ops


def _components(ops):
    """Union output runs (o, d) into overlap-connected components."""
    runs = sorted(set((o, d) for (_, _, o, d, _) in ops))
    parent = {r: r for r in runs}

    def find(r):
        while parent[r] != r:
            parent[r] = parent[parent[r]]
            r = parent[r]
        return r

    for i, ri in enumerate(runs):
        for rj in runs[i+1:]:
            if ri[0] < rj[0] + rj[1] and rj[0] < ri[0] + ri[1]:
                ra, rb = find(ri), find(rj)
                if ra != rb:
                    parent[ra] = rb
    comp_of = {r: find(r) for r in runs}
    # canonical comp descriptor: (lo, hi)
    spans = {}
    for r in runs:
        c = comp_of[r]
        lo, hi = spans.get(c, (10**9, -1))
        spans[c] = (min(lo, r[0]), max(hi, r[0] + r[1]))
    return {r: spans[comp_of[r]] for r in runs}


def _build_plan(cg, r1, r2, ro, out_dim, pe_cap=PE_CAP):
    ops = _extract_sliceops(cg, r1, r2, ro)

    # ---- shared products -------------------------------------------------
    pair_order, pair_idx = [], {}
    for (a, b, o, d, c) in ops:
        key = (a, b, d)
        if key not in pair_idx:
            pair_idx[key] = len(pair_order)
            pair_order.append(key)
    slot, cur = {}, 0
    for key in pair_order:
        slot[key] = cur
        cur += key[2]
    psize = cur

    prod_instrs = []
    i = 0
    while i < len(pair_order):
        a0, b0, d0 = pair_order[i]
        s0 = slot[pair_order[i]]
        j = i + 1
        da = db = ds = None
        while j < len(pair_order):
            a1, b1, d1 = pair_order[j]
            if d1 != d0:
                break
            nda = a1 - pair_order[j-1][0]
            ndb = b1 - pair_order[j-1][1]
            nds = slot[pair_order[j]] - slot[pair_order[j-1]]
            if da is None:
                da, db, ds = nda, ndb, nds
            elif (nda, ndb, nds) != (da, db, ds):
                break
            j += 1
        n = j - i
        if n == 1:
            da = db = ds = 0
        prod_instrs.append(dict(pslot=s0, a=a0, b=b0, d=d0,
                                da=da, db=db, ds=ds, n=n))
        i = j

    # ---- accumulation ops with component labels --------------------------
    comp_of = _components(ops)
    accs = [dict(o=o, pslot=slot[(a, b, d)], c=c, d=d, comp=comp_of[(o, d)])
            for (a, b, o, d, c) in ops]

    comps = sorted(set(q['comp'] for q in accs))
    comp_elems = {cm: 0 for cm in comps}
    comp_nops = {cm: 0 for cm in comps}
    for q in accs:
        comp_elems[q['comp']] += q['d']
        comp_nops[q['comp']] += 1

    # ---- engine assignment ----------------------------------------------
    # PE takes the densest components (accum elems per output position)
    # until the PSUM slot budget is filled; leftovers balance DVE/Pool.
    prod_elems_total = sum(p['n'] * p['d'] for p in prod_instrs)
    pe_width = 0
    comp_eng = {}
    for cm in sorted(comps, key=lambda cm: -comp_elems[cm] / (cm[1] - cm[0])):
        w = cm[1] - cm[0]
        if pe_width + w <= pe_cap:
            comp_eng[cm] = 'pe'
            pe_width += w
    busy = {'dve': 1500.0 + prod_elems_total * RATE_DVE_TT, 'pool': 0.0}
    for cm in sorted(comps, key=lambda cm: -comp_elems[cm]):
        if cm in comp_eng:
            continue
        elems = comp_elems[cm]
        td = busy['dve'] + elems * RATE_DVE_STT
        tp = busy['pool'] + elems * RATE_POOL_STT
        if td <= tp:
            comp_eng[cm] = 'dve'
            busy['dve'] = td
        else:
            comp_eng[cm] = 'pool'
            busy['pool'] = tp

    # products all on DVE: a second producer engine would add a semaphore
    # wait to every consumer's first instruction (walrus wait-slot limits)
    for pi in prod_instrs:
        pi['eng'] = 'dve'

    # ---- PSUM bank packing (span order, first-fit into 128-slot banks) ---
    n_banks = (pe_cap + BANK_SLOTS - 1) // BANK_SLOTS
    pe_comps = [cm for cm in comps if comp_eng[cm] == 'pe']
    bank_of, base_of = {}, {}
    bank_fill = [0] * n_banks
    for cm in pe_comps:                      # span order keeps evicts merged
        w = cm[1] - cm[0]
        for b in range(n_banks):
            if bank_fill[b] + w <= BANK_SLOTS:
                bank_of[cm], base_of[cm] = b, bank_fill[b]
                bank_fill[b] += w
                break
        else:
            raise RuntimeError("PSUM bank packing failed")

    # eviction instructions: maximal runs of PE comps, contiguous in both
    # psum slots and output positions
    evicts = []
    for cm in pe_comps:
        b, s, o, w = bank_of[cm], base_of[cm], cm[0], cm[1] - cm[0]
        if (evicts and evicts[-1]['bank'] == b
                and evicts[-1]['slot'] + evicts[-1]['w'] == s
                and evicts[-1]['o'] + evicts[-1]['w'] == o):
            evicts[-1]['w'] += w
        else:
            evicts.append(dict(bank=b, slot=s, o=o, w=w))

    # ---- per-engine accumulation instructions ----------------------------
    # PE: psum slot per op; merge same-(c,bank) contiguous ops, cap w<=128
    pe_ops = []
    for q in accs:
        cm = q['comp']
        if comp_eng[cm] != 'pe':
            continue
        pe_ops.append(dict(bank=bank_of[cm], slot=base_of[cm] + q['o'] - cm[0],
                           pslot=q['pslot'], c=q['c'], d=q['d'], o=q['o']))
    # PSUM cells must be uniformly first-write or all-accumulate within one
    # matmul: split ops into a first-touch phase (covers every cell once,
    # wide-first) and an accumulate phase; c-grouped within each phase.
    covered = np.zeros(out_dim, bool)
    for q in sorted(pe_ops, key=lambda q: (-q['d'], q['c'], q['o'])):
        rng = slice(q['o'], q['o'] + q['d'])
        if not covered[rng].any():
            q['ft'] = True
        else:
            assert covered[rng].all(), "partial first-touch in PE domain"
            q['ft'] = False
        covered[rng] = True
    cmin = {}
    for q in pe_ops:
        cmin[q['c']] = min(cmin.get(q['c'], 1 << 30), q['pslot'])
    pe_ops.sort(key=lambda q: (not q['ft'], cmin[q['c']], q['c'], q['pslot'],
                               q['bank'], q['slot']))
    pe_instrs = []
    i = 0
    while i < len(pe_ops):
        q0 = pe_ops[i]
        j = i + 1
        while j < len(pe_ops):
            q1, qp = pe_ops[j], pe_ops[j-1]
            if (q1['c'] != q0['c'] or q1['bank'] != q0['bank']
                    or q1['d'] != q0['d'] or q1['ft'] != q0['ft']
                    or q1['slot'] - qp['slot'] != q0['d']
                    or q1['pslot'] - qp['pslot'] != q0['d']
                    or (j - i + 1) * q0['d'] > BANK_SLOTS):
                break
            j += 1
        n = j - i
        pe_instrs.append(dict(bank=q0['bank'], slot=q0['slot'], pslot=q0['pslot'],
                              c=q0['c'], w=n * q0['d']))
        i = j
    # stop flags: last instruction per bank in emission order.  Banks are
    # OPENED by a sacrificial zero-add matmul (start_tensor_calc zeros the
    # bank and absorbs the PSUM write-after-write wait), so no real
    # instruction carries start.
    last_idx = {}
    for idx, q in enumerate(pe_instrs):
        q['start'] = False
        last_idx[q['bank']] = idx
        q['stop'] = False
    for b, idx in last_idx.items():
        pe_instrs[idx]['stop'] = True
    bank_stop = {b: (pe_instrs[i]['slot'], pe_instrs[i]['w'])
                 for b, i in last_idx.items()}

    # DVE/Pool: TS first-touch then STT, merged contiguous same-c runs
    sb_instrs = []
    for eng in ('dve', 'pool'):
        qs = [q for q in accs if comp_eng[q['comp']] == eng]
        covered = np.zeros(out_dim, bool)
        qs.sort(key=lambda q: (-q['d'], q['c'], q['o'], q['pslot']))
        for q in qs:
            rng = slice(q['o'], q['o'] + q['d'])
            if not covered[rng].any():
                q['kind'] = 'TS'
            else:
                assert covered[rng].all(), "partial first-touch"
                q['kind'] = 'STT'
            covered[rng] = True
        qs.sort(key=lambda q: (q['kind'] != 'TS', -q['d'], q['c'], q['o'],
                               q['pslot']))
        i = 0
        while i < len(qs):
            q0 = qs[i]
            j = i + 1
            while j < len(qs):
                q1, qp = qs[j], qs[j-1]
                if (q1['kind'] != q0['kind'] or q1['d'] != q0['d']
                        or q1['c'] != q0['c']
                        or q1['o'] - qp['o'] != q0['d']
                        or q1['pslot'] - qp['pslot'] != q0['d']):
                    break
                j += 1
            n = j - i
            sb_instrs.append(dict(kind=q0['kind'], o=q0['o'], pslot=q0['pslot'],
                                  c=q0['c'], d=q0['d'], n=n, eng=eng))
            i = j

    cvals = sorted(set(q['c'] for q in pe_instrs))
    last_pi = prod_instrs[-1]
    return dict(psize=psize, prod_instrs=prod_instrs, pe_instrs=pe_instrs,
                sb_instrs=sb_instrs, evicts=evicts, cvals=cvals,
                n_banks=n_banks, xl1=last_pi['a'], xl2=last_pi['b'],
                bank_stop=bank_stop)


# ----------------------------------------------------------------------------
# Bass program
# ----------------------------------------------------------------------------

def _build_bass(plan, rows_per_core, rep_dim, out_dim, repeat=1):
    import concourse.bass as bass
    import concourse.mybir as mybir
    from concourse.ap import AP
    from concourse.tile import TileContext
    import concourse.tile as _tile_mod
    from concourse.vector_clock import ScopedClock as _ScopedClock

    # The kernel-tail Drain instruction waits on every proc lane with
    # outstanding ticks, but its CTRL ISA struct only has room for a couple
    # of embedded sync-wait commands ("Too many sync wait commands" in
    # walrus codegen otherwise).  Split the global-clock wait across
    # several Drain instructions (waits already observed by the SP engine
    # are elided by add_sem_waits).
    if not getattr(_tile_mod.TileContext, '_cg_drain_patched', False):
        def _split_drain_and_barrier(self, tick_clock, wait_clock):
            gc = tick_clock.global_clock
            VC = type(gc)
            procs = []
            for p in range(27):
                t = gc.peek_next(p) - 1
                if t > 0:
                    procs.append((p, t))
            for i in range(0, len(procs), 1):
                pc = VC()
                for p, t in procs[i:i + 1]:
                    for _ in range(t):
                        pc.advance(p)
                d = self.nc.sync.drain()
                wait_clock.add_sem_waits(d.ins, _ScopedClock({None: pc}))
            self.nc.all_engine_barrier()
            popped = self.nc._tile_sem_poison_stack.pop()
            assert popped is self._sem_poison
            self.nc.clear_and_free_semaphores(list(self.sems.allocated().values()))
            self.nc.all_engine_barrier()

        _tile_mod.TileContext._drain_and_barrier = _split_drain_and_barrier
        _tile_mod.TileContext._cg_drain_patched = True

    f16 = mybir.dt.float16
    f32 = mybir.dt.float32
    T = T_FOLD
    n_groups = rows_per_core // (P_DIM * T)
    assert rows_per_core == n_groups * P_DIM * T

    nc = bass.Bass("TRN2")
    x1d = nc.declare_dram_parameter("x1", [rows_per_core, rep_dim], f16, isOutput=False)
    x2d = nc.declare_dram_parameter("x2", [rows_per_core, rep_dim], f16, isOutput=False)
    outd = nc.declare_dram_parameter("out", [rows_per_core, out_dim], f16, isOutput=True)

    def ap_custom(tile, base, dims):
        a = tile[:]
        aplist = [list(a.ap[0])] + [[s, n] for (s, n) in dims]
        return AP(a.tensor, a.offset + base, aplist)

    cvals = plan['cvals']
    with TileContext(nc) as tc:
        with (
            tc.tile_pool(name="const", bufs=1) as cstp,
            tc.tile_pool(name="io", bufs=4) as iop,
            tc.tile_pool(name="pp", bufs=4) as ppp,
            tc.tile_pool(name="ps", bufs=2, space="PSUM") as psp,
        ):
            # one-time: diag(c) stationary tiles built from an identity
            IDT = cstp.tile([P_DIM, P_DIM], f16, tag="IDT")
            nc.gpsimd.memset(IDT[:], 1.0)
            nc.gpsimd.affine_select(
                IDT[:], IDT[:], pattern=[[1, P_DIM]],
                compare_op=mybir.AluOpType.is_equal, fill=0.0,
                base=0, channel_multiplier=-1)
            DIAG = {}
            for c in cvals:
                D = cstp.tile([P_DIM, P_DIM], f16, tag=f"D{len(DIAG)}")
                nc.scalar.mul(D[:], IDT[:], float(c))
                DIAG[c] = D
            ZT = cstp.tile([P_DIM, BANK_F32], f16, tag="ZT")
            nc.gpsimd.memset(ZT[:], 0.0)
            CZ = cstp.tile([P_DIM, 2], f16, tag="CZ")
            nc.vector.memset(CZ[:], 0.0)

            def dram_group_ap(dram, g, width):
                # [128p, T, width] view of rows [g*T*128, (g+1)*T*128)
                a = dram[:]
                return AP(a.tensor, g * T * P_DIM * width,
                          [[width, P_DIM], [P_DIM * width, T], [1, width]])

            csz = plan['psize']
            pool_qs = [q for q in plan['sb_instrs'] if q['eng'] == 'pool']
            P_hist = []
            for it in range(n_groups * repeat):
                g = it % n_groups
                warm = it >= 1   # a previous iteration exists
                X1 = iop.tile([P_DIM, T * rep_dim], f16, tag="X1")
                X2 = iop.tile([P_DIM, T * rep_dim], f16, tag="X2")
                O = iop.tile([P_DIM, T * out_dim], f16, tag="O")
                PS = psp.tile([P_DIM, plan['n_banks'] * BANK_F32], f32, tag="PS")
                # scratch tiles are single-engine or pairwise so the
                # first toucher of a recycled buffer inherits at most one
                # foreign engine's tile-reuse wait
                SCD = iop.tile([P_DIM, 8], f16, tag="SCD")    # DVE only
                SDA = iop.tile([P_DIM, 2], f16, tag="SDA")    # DVE -> Act
                SPD = iop.tile([P_DIM, 2], f16, tag="SPD")    # Pool -> DVE
                SED = iop.tile([P_DIM, 4], f16, tag="SED")    # DVE -> PE
                SEA = iop.tile([P_DIM, 2], f16, tag="SEA")    # Act -> PE
                P = ppp.tile([P_DIM, T * csz], f16, tag="P")

                # Every ISA struct holds only 1-2 embedded sync waits, and
                # the tile framework expands transitive vector clocks into
                # explicit waits.  Each engine therefore "observes the
                # world" through a chain of absorber instructions, each
                # adding at most ~2 new clock components, so the hot
                # instructions carry at most one wait.
                # Pool marker: observes the DVE products that wrote this
                # P buffer 4 iterations ago, so the input DMAs' WAR waits
                # elide (the DMA then carries only its queue sem).
                # Pool markers write into X1/X2 at the bytes the last
                # product reads: they are the first toucher of the recycled
                # input tiles, absorbing the DVE write-after-read waits;
                # nosync deps keep the DMA prefetch pass behind them.
                markers = []
                if it >= 4:
                    markers.append(nc.gpsimd.tensor_copy(
                        X1[:, plan['xl1']:plan['xl1'] + 2], CZ[:, 0:2]))
                    markers.append(nc.gpsimd.tensor_copy(
                        X2[:, plan['xl2']:plan['xl2'] + 2], CZ[:, 0:2]))
                if it >= 4:
                    nc.gpsimd.tensor_copy(
                        SPD[:, 0:2], P_hist[-4][:, T * csz - 2:T * csz])
                d1 = nc.gpsimd.dma_start(X1[:], dram_group_ap(x1d, g, rep_dim))
                d2 = nc.gpsimd.dma_start(X2[:], dram_group_ap(x2d, g, rep_dim))
                if markers:
                    from concourse.bass import InstructionNameOrderedSet
                    dep = InstructionNameOrderedSet()
                    for m in markers:
                        dep.add(m.ins.name)
                    d1.ins.add_nosync_dependencies_from(dep)
                    d2.ins.add_nosync_dependencies_from(dep)
                # DVE absorbers: DMA completion sems, Pool's marker, and the
                # PE ticks of 2 iterations ago (via a PSUM read of the last
                # bank-stop range) so the products carry no waits
                nc.vector.tensor_copy(SDA[:, 0:2], X1[:, 0:2])
                nc.vector.tensor_copy(SCD[:, 0:2], X2[:, 0:2])
                if it >= 4:
                    nc.vector.tensor_copy(SCD[:, 2:4], SPD[:, 0:2])
                if it >= 4:
                    bl = plan['pe_instrs'][-1]
                    ps_off = (bl['bank'] * BANK_F32 + (T - 1) * BANK_SLOTS
                              + bl['slot'] + bl['w'] - 2)
                    nc.vector.tensor_copy(SED[:, 0:2],
                                          ap_custom(PS, ps_off, [(1, 2)]))
                nc.vector.tensor_copy(SED[:, 2:4], SDA[:, 0:2])
                # Act observes its own PE-pair tile, then DVE: two
                # single-wait instructions
                if it >= 4:
                    nc.scalar.copy(SEA[:, 0:2], SEA[:, 0:2])
                nc.scalar.copy(SEA[:, 0:2], SDA[:, 0:2])

                for pi in plan['prod_instrs']:
                    dims = [(csz, T), (pi['ds'], pi['n']), (1, pi['d'])]
                    eng = nc.vector if pi['eng'] == 'dve' else nc.gpsimd
                    eng.tensor_tensor(
                        ap_custom(P, pi['pslot'], dims),
                        ap_custom(X1, pi['a'],
                                  [(rep_dim, T), (pi['da'], pi['n']), (1, pi['d'])]),
                        ap_custom(X2, pi['b'],
                                  [(rep_dim, T), (pi['db'], pi['n']), (1, pi['d'])]),
                        mybir.AluOpType.mult,
                    )

                # PE absorbers: garbage ldweights reading the observer
                # scratch; each pulls in the producer's clock so the real
                # matmuls carry at most one wait.  The first real ldweights
                # overwrites the PE array anyway.
                nc.tensor.ldweights(SED[:, 2:4])
                if it >= 4:
                    nc.tensor.ldweights(SED[:, 0:2])
                nc.tensor.ldweights(SEA[:, 0:2])

                # PE scatter-add: diag(c) matmuls accumulating in PSUM.
                # One matmul per row-tile t: a single matmul's PSUM out AP
                # must be flat and stay within one 2KB bank.
                c0 = cvals[0]
                for b in range(plan['n_banks']):
                    out_ap = ap_custom(PS, b * BANK_F32, [(1, BANK_F32)])
                    nc.tensor.matmul(out_ap, DIAG[c0][:], ZT[:],
                                     start=True, stop=False)
                for qi in plan['pe_instrs']:
                    for t in range(T):
                        out_ap = ap_custom(
                            PS, qi['bank'] * BANK_F32 + t * BANK_SLOTS + qi['slot'],
                            [(1, qi['w'])])
                        mov_ap = ap_custom(P, t * csz + qi['pslot'],
                                           [(1, qi['w'])])
                        nc.tensor.matmul(out_ap, DIAG[qi['c']][:], mov_ap,
                                         start=False,
                                         stop=qi['stop'] and t == T - 1)

                # DVE/Pool: leftover components in SBUF.  Per-engine
                # absorbers: read the LAST product's range (observes the max
                # DVE tick -> later product-RAW waits elide) and touch O to
                # absorb the WAR against the out-DMA two groups back.
                nc.vector.tensor_copy(SCD[:, 6:8],
                                      P[:, T * csz - 2:T * csz])
                if it >= 4:
                    seen_comp = set()
                    for qi in plan['sb_instrs']:
                        cmk = (qi['eng'], qi['o'] // 32)
                        if cmk in seen_comp:
                            continue
                        seen_comp.add(cmk)
                        eng = nc.vector if qi['eng'] == 'dve' else nc.gpsimd
                        eng.tensor_copy(ap_custom(O, qi['o'], [(1, 2)]),
                                        SCD[:, 6:8])
                for qi in plan['sb_instrs']:
                    w = qi['n'] * qi['d']
                    o_ap = ap_custom(O, qi['o'], [(out_dim, T), (1, w)])
                    p_ap = ap_custom(P, qi['pslot'], [(csz, T), (1, w)])
                    eng = nc.vector if qi['eng'] == 'dve' else nc.gpsimd
                    if qi['kind'] == 'TS':
                        eng.tensor_scalar_mul(o_ap, p_ap, float(qi['c']))
                    else:
                        eng.scalar_tensor_tensor(
                            out=o_ap, in0=p_ap, scalar=float(qi['c']),
                            in1=o_ap,
                            op0=mybir.AluOpType.mult,
                            op1=mybir.AluOpType.add,
                        )

                # Act: absorb the out-DMA(g-2) WAR once, then evict
                # PSUM -> fp16 output tile (single PE wait each)
                if it >= 4:
                    for ev in plan['evicts']:
                        nc.scalar.copy(ap_custom(O, ev['o'], [(1, 2)]),
                                       SDA[:, 0:2])
                for ev in plan['evicts']:
                    src = ap_custom(PS, ev['bank'] * BANK_F32 + ev['slot'],
                                    [(BANK_SLOTS, T), (1, ev['w'])])
                    dst = ap_custom(O, ev['o'], [(out_dim, T), (1, ev['w'])])
                    nc.scalar.copy(dst, src)
                # Act gates the non-Act writers of O (DVE accums) with an
                # in-place 2-element copy carrying that single wait, then
                # issues the out-DMA itself: all O writers are then its own
                # in-order engine, leaving only the DMA queue wait.
                dve_qs = [q for q in plan['sb_instrs'] if q['eng'] == 'dve']
                if dve_qs:
                    ql = dve_qs[-1]
                    ap = ap_custom(O, ql['o'] + ql['n'] * ql['d'] - 2, [(1, 2)])
                    nc.scalar.copy(ap, ap)
                nc.scalar.dma_start(dram_group_ap(outd, g, out_dim), O[:])
                P_hist.append(P)
    return nc


# ----------------------------------------------------------------------------
# Entry point
# ----------------------------------------------------------------------------

def kernel(x1, x2, cg_tilde, repids_in1, repids_in2, repids_out, out_dim):
    from concourse.bass_utils import run_bass_kernel_spmd

    x1 = np.asarray(x1, dtype=np.float16)
    x2 = np.asarray(x2, dtype=np.float16)
    cg = np.asarray(cg_tilde, dtype=np.float32)
    r1 = np.asarray(repids_in1).astype(np.int64)
    r2 = np.asarray(repids_in2).astype(np.int64)
    ro = np.asarray(repids_out).astype(np.int64)
    out_dim = int(out_dim)

    n, rep_dim = x1.shape
    rows_per_core = n // N_CORES

    key = (rows_per_core, rep_dim, out_dim, cg.tobytes(), r1.tobytes(),
           r2.tobytes(), ro.tobytes())
    cache_key = hash(key)
    if cache_key not in _BUILD_CACHE:
        plan = _build_plan(cg, r1, r2, ro, out_dim)
        nc = _build_bass(plan, rows_per_core, rep_dim, out_dim)
        _BUILD_CACHE[cache_key] = nc
    nc = _BUILD_CACHE[cache_key]

    in_maps = [
        {"x1": x1[i*rows_per_core:(i+1)*rows_per_core],
         "x2": x2[i*rows_per_core:(i+1)*rows_per_core]}
        for i in range(N_CORES)
    ]
    res = run_bass_kernel_spmd(nc, in_maps, list(range(N_CORES)))
    out = np.concatenate([res.results[i]["out"] for i in range(N_CORES)], axis=0)
    return out.astype(np.float32)


# revision 57
# speedup vs baseline: 41.4291x; 10.6884x over previous
"""CGCoupler Trainium2 Bass kernel.

out[n, ro[k]] += x1[n, r1[k]] * x2[n, r2[k]] * cg[k]  for all k, rows n.

Because the CG index tables address contiguous channel runs, the whole op
decomposes into ~147 contiguous-slice FMAs per row:
    out[:, o:o+d] += c * x1[:, a:a+d] * x2[:, b:b+d]
with d in {32, 64}.  Rows live on the 128 SBUF partitions, the 640-wide
feature dim on the free axis, T=4 row-tiles folded per instruction.

All device data is fp16 (inputs cast on host, output cast back): halves
HBM traffic and enables the DVE 2x perf mode for the TensorTensor
products.  The accumulation work is spread across engines:

  * DVE computes the shared products P = x1*x2 (70 unique slice pairs).
  * PE (tensor engine) does most of the scatter-add: matmul with a
    diag(c) stationary is a scaled copy-accumulate into PSUM, so each
    CG term is one matmul; PSUM accumulation is free and exact (fp32).
    Output runs are packed into PSUM banks (one 2KB bank = 128 output
    positions x T row-tiles); the first matmul per bank sets
    start_tensor_calc (hardware zeros the bank), the last sets stop.
  * Act (scalar engine) evicts PSUM -> fp16 output tile.
  * Pool (gpsimd) takes the leftover components as scalar_tensor_tensor
    accumulations directly in SBUF.

Data-parallel across 8 NeuronCores: each core processes 2048 rows.
"""
import numpy as np

N_CORES = 8
P_DIM = 128
T_FOLD = 4            # row-tiles folded per instruction group
BANK_F32 = 512        # PSUM bank capacity in fp32 elements
BANK_SLOTS = BANK_F32 // T_FOLD   # output positions per PSUM bank
N_PE_BANKS = 4        # banks per PSUM buffer (x2 buffers = all 8)

# cost-model ns-per-(row-elem) rates (x16 free-elems per core) for balancing
RATE_PE = 6.7
RATE_POOL_STT = 1e9
RATE_DVE_STT = 17.3
RATE_DVE_TT = 10.8
RATE_POOL_TT = 35.5
PE_CAP = 512          # PSUM slots available to PE-assigned output positions

_BUILD_CACHE = {}


# ----------------------------------------------------------------------------
# Planning
# ----------------------------------------------------------------------------

def _extract_sliceops(cg, r1, r2, ro):
    M = len(cg)
    ops = []
    k = 0
    while k < M:
        j = k + 1
        while (j < M and r1[j] == r1[j-1] + 1 and r2[j] == r2[j-1] + 1
               and ro[j] == ro[j-1] + 1 and cg[j] == cg[k]):
            j += 1
        ops.append((int(r1[k]), int(r2[k]), int(ro[k]), j - k, float(cg[k])))
        k = j
    return ops


def _components(ops):
    """Union output runs (o, d) into overlap-connected components."""
    runs = sorted(set((o, d) for (_, _, o, d, _) in ops))
    parent = {r: r for r in runs}

    def find(r):
        while parent[r] != r:
            parent[r] = parent[parent[r]]
            r = parent[r]
        return r

    for i, ri in enumerate(runs):
        for rj in runs[i+1:]:
            if ri[0] < rj[0] + rj[1] and rj[0] < ri[0] + ri[1]:
                ra, rb = find(ri), find(rj)
                if ra != rb:
                    parent[ra] = rb
    comp_of = {r: find(r) for r in runs}
    # canonical comp descriptor: (lo, hi)
    spans = {}
    for r in runs:
        c = comp_of[r]
        lo, hi = spans.get(c, (10**9, -1))
        spans[c] = (min(lo, r[0]), max(hi, r[0] + r[1]))
    return {r: spans[comp_of[r]] for r in runs}


def _build_plan(cg, r1, r2, ro, out_dim, pe_cap=PE_CAP):
    ops = _extract_sliceops(cg, r1, r2, ro)

    # ---- shared products -------------------------------------------------
    pair_order, pair_idx = [], {}
    for (a, b, o, d, c) in ops:
        key = (a, b, d)
        if key not in pair_idx:
            pair_idx[key] = len(pair_order)
            pair_order.append(key)
    # sort pairs by (d, a, b): adjacent pairs then mostly advance with
    # constant strides, merging 70 pairs into ~15 product instructions
    pair_order.sort(key=lambda p: (p[2], p[0], p[1]))
    slot, cur = {}, 0
    for key in pair_order:
        slot[key] = cur
        cur += key[2]
    psize = cur

    prod_instrs = []
    i = 0
    while i < len(pair_order):
        a0, b0, d0 = pair_order[i]
        s0 = slot[pair_order[i]]
        j = i + 1
        da = db = ds = None
        while j < len(pair_order):
            a1, b1, d1 = pair_order[j]
            if d1 != d0:
                break
            nda = a1 - pair_order[j-1][0]
            ndb = b1 - pair_order[j-1][1]
            nds = slot[pair_order[j]] - slot[pair_order[j-1]]
            if da is None:
                da, db, ds = nda, ndb, nds
            elif (nda, ndb, nds) != (da, db, ds):
                break
            j += 1
        n = j - i
        if n == 1:
            da = db = ds = 0
        prod_instrs.append(dict(pslot=s0, a=a0, b=b0, d=d0,
                                da=da, db=db, ds=ds, n=n))
        i = j

    # ---- accumulation ops with component labels --------------------------
    comp_of = _components(ops)
    accs = [dict(o=o, pslot=slot[(a, b, d)], c=c, d=d, comp=comp_of[(o, d)])
            for (a, b, o, d, c) in ops]

    comps = sorted(set(q['comp'] for q in accs))
    comp_elems = {cm: 0 for cm in comps}
    comp_nops = {cm: 0 for cm in comps}
    for q in accs:
        comp_elems[q['comp']] += q['d']
        comp_nops[q['comp']] += 1

    # ---- engine assignment ----------------------------------------------
    # PE takes the densest components (accum elems per output position)
    # until the PSUM slot budget is filled; leftovers balance DVE/Pool.
    prod_elems_total = sum(p['n'] * p['d'] for p in prod_instrs)
    pe_width = 0
    comp_eng = {}
    for cm in sorted(comps, key=lambda cm: -comp_elems[cm] / (cm[1] - cm[0])):
        w = cm[1] - cm[0]
        if pe_width + w <= pe_cap:
            comp_eng[cm] = 'pe'
            pe_width += w
    busy = {'dve': 1500.0 + prod_elems_total * RATE_DVE_TT, 'pool': 0.0}
    for cm in sorted(comps, key=lambda cm: -comp_elems[cm]):
        if cm in comp_eng:
            continue
        elems = comp_elems[cm]
        td = busy['dve'] + elems * RATE_DVE_STT
        tp = busy['pool'] + elems * RATE_POOL_STT
        if td <= tp:
            comp_eng[cm] = 'dve'
            busy['dve'] = td
        else:
            comp_eng[cm] = 'pool'
            busy['pool'] = tp

    # products all on DVE: a second producer engine would add a semaphore
    # wait to every consumer's first instruction (walrus wait-slot limits)
    for pi in prod_instrs:
        pi['eng'] = 'dve'

    # ---- PSUM bank packing (span order, first-fit into 128-slot banks) ---
    n_banks = (pe_cap + BANK_SLOTS - 1) // BANK_SLOTS
    pe_comps = [cm for cm in comps if comp_eng[cm] == 'pe']
    bank_of, base_of = {}, {}
    bank_fill = [0] * n_banks
    for cm in pe_comps:                      # span order keeps evicts merged
        w = cm[1] - cm[0]
        for b in range(n_banks):
            if bank_fill[b] + w <= BANK_SLOTS:
                bank_of[cm], base_of[cm] = b, bank_fill[b]
                bank_fill[b] += w
                break
        else:
            raise RuntimeError("PSUM bank packing failed")

    # eviction instructions: maximal runs of PE comps, contiguous in both
    # psum slots and output positions
    evicts = []
    for cm in pe_comps:
        b, s, o, w = bank_of[cm], base_of[cm], cm[0], cm[1] - cm[0]
        if (evicts and evicts[-1]['bank'] == b
                and evicts[-1]['slot'] + evicts[-1]['w'] == s
                and evicts[-1]['o'] + evicts[-1]['w'] == o):
            evicts[-1]['w'] += w
        else:
            evicts.append(dict(bank=b, slot=s, o=o, w=w))

    # ---- per-engine accumulation instructions ----------------------------
    # PE: psum slot per op; merge same-(c,bank) contiguous ops, cap w<=128
    pe_ops = []
    for q in accs:
        cm = q['comp']
        if comp_eng[cm] != 'pe':
            continue
        pe_ops.append(dict(bank=bank_of[cm], slot=base_of[cm] + q['o'] - cm[0],
                           pslot=q['pslot'], c=q['c'], d=q['d'], o=q['o']))
    # PSUM cells must be uniformly first-write or all-accumulate within one
    # matmul: split ops into a first-touch phase (covers every cell once,
    # wide-first) and an accumulate phase; c-grouped within each phase.
    covered = np.zeros(out_dim, bool)
    for q in sorted(pe_ops, key=lambda q: (-q['d'], q['c'], q['o'])):
        rng = slice(q['o'], q['o'] + q['d'])
        if not covered[rng].any():
            q['ft'] = True
        else:
            assert covered[rng].all(), "partial first-touch in PE domain"
            q['ft'] = False
        covered[rng] = True
    cmin = {}
    for q in pe_ops:
        cmin[q['c']] = min(cmin.get(q['c'], 1 << 30), q['pslot'])
    pe_ops.sort(key=lambda q: (not q['ft'], cmin[q['c']], q['c'], q['pslot'],
                               q['bank'], q['slot']))
    pe_instrs = []
    i = 0
    while i < len(pe_ops):
        q0 = pe_ops[i]
        j = i + 1
        while j < len(pe_ops):
            q1, qp = pe_ops[j], pe_ops[j-1]
            if (q1['c'] != q0['c'] or q1['bank'] != q0['bank']
                    or q1['d'] != q0['d'] or q1['ft'] != q0['ft']
                    or q1['slot'] - qp['slot'] != q0['d']
                    or q1['pslot'] - qp['pslot'] != q0['d']
                    or (j - i + 1) * q0['d'] > BANK_SLOTS):
                break
            j += 1
        n = j - i
        pe_instrs.append(dict(bank=q0['bank'], slot=q0['slot'], pslot=q0['pslot'],
                              c=q0['c'], w=n * q0['d']))
        i = j
    # stop flags: last instruction per bank in emission order.  Banks are
    # OPENED by a sacrificial zero-add matmul (start_tensor_calc zeros the
    # bank and absorbs the PSUM write-after-write wait), so no real
    # instruction carries start.
    last_idx = {}
    for idx, q in enumerate(pe_instrs):
        q['start'] = False
        last_idx[q['bank']] = idx
        q['stop'] = False
    for b, idx in last_idx.items():
        pe_instrs[idx]['stop'] = True
    bank_stop = {b: (pe_instrs[i]['slot'], pe_instrs[i]['w'])
                 for b, i in last_idx.items()}

    # DVE/Pool: TS first-touch then STT, merged contiguous same-c runs
    sb_instrs = []
    for eng in ('dve', 'pool'):
        qs = [q for q in accs if comp_eng[q['comp']] == eng]
        covered = np.zeros(out_dim, bool)
        qs.sort(key=lambda q: (-q['d'], q['c'], q['o'], q['pslot']))
        for q in qs:
            rng = slice(q['o'], q['o'] + q['d'])
            if not covered[rng].any():
                q['kind'] = 'TS'
            else:
                assert covered[rng].all(), "partial first-touch"
                q['kind'] = 'STT'
            covered[rng] = True
        qs.sort(key=lambda q: (q['kind'] != 'TS', -q['d'], q['c'], q['o'],
                               q['pslot']))
        i = 0
        while i < len(qs):
            q0 = qs[i]
            j = i + 1
            while j < len(qs):
                q1, qp = qs[j], qs[j-1]
                if (q1['kind'] != q0['kind'] or q1['d'] != q0['d']
                        or q1['c'] != q0['c']
                        or q1['o'] - qp['o'] != q0['d']
                        or q1['pslot'] - qp['pslot'] != q0['d']):
                    break
                j += 1
            n = j - i
            sb_instrs.append(dict(kind=q0['kind'], o=q0['o'], pslot=q0['pslot'],
                                  c=q0['c'], d=q0['d'], n=n, eng=eng))
            i = j

    asize = 0
    for q in sb_instrs:
        if q['eng'] == 'dve' and q['kind'] == 'STT':
            q['aslot'] = asize
            asize += q['n'] * q['d']

    cvals = sorted(set(q['c'] for q in pe_instrs))
    last_pi = prod_instrs[-1]
    return dict(psize=psize, prod_instrs=prod_instrs, pe_instrs=pe_instrs,
                sb_instrs=sb_instrs, evicts=evicts, cvals=cvals,
                n_banks=n_banks, xl1=last_pi['a'], xl2=last_pi['b'],
                bank_stop=bank_stop, asize=asize)


# ----------------------------------------------------------------------------
# Bass program
# ----------------------------------------------------------------------------

def _build_bass(plan, rows_per_core, rep_dim, out_dim, repeat=1):
    import concourse.bass as bass
    import concourse.mybir as mybir
    from concourse.ap import AP
    from concourse.tile import TileContext
    import concourse.tile as _tile_mod
    from concourse.vector_clock import ScopedClock as _ScopedClock

    # The kernel-tail Drain instruction waits on every proc lane with
    # outstanding ticks, but its CTRL ISA struct only has room for a couple
    # of embedded sync-wait commands ("Too many sync wait commands" in
    # walrus codegen otherwise).  Split the global-clock wait across
    # several Drain instructions (waits already observed by the SP engine
    # are elided by add_sem_waits).
    if not getattr(_tile_mod.TileContext, '_cg_drain_patched', False):
        def _split_drain_and_barrier(self, tick_clock, wait_clock):
            gc = tick_clock.global_clock
            VC = type(gc)
            procs = []
            for p in range(27):
                t = gc.peek_next(p) - 1
                if t > 0:
                    procs.append((p, t))
            for i in range(0, len(procs), 1):
                pc = VC()
                for p, t in procs[i:i + 1]:
                    for _ in range(t):
                        pc.advance(p)
                d = self.nc.sync.drain()
                wait_clock.add_sem_waits(d.ins, _ScopedClock({None: pc}))
            self.nc.all_engine_barrier()
            popped = self.nc._tile_sem_poison_stack.pop()
            assert popped is self._sem_poison
            self.nc.clear_and_free_semaphores(list(self.sems.allocated().values()))
            self.nc.all_engine_barrier()

        _tile_mod.TileContext._drain_and_barrier = _split_drain_and_barrier
        _tile_mod.TileContext._cg_drain_patched = True

    f16 = mybir.dt.float16
    f32 = mybir.dt.float32
    T = T_FOLD
    n_groups = rows_per_core // (P_DIM * T)
    assert rows_per_core == n_groups * P_DIM * T

    nc = bass.Bass("TRN2")
    x1d = nc.declare_dram_parameter("x1", [rows_per_core, rep_dim], f16, isOutput=False)
    x2d = nc.declare_dram_parameter("x2", [rows_per_core, rep_dim], f16, isOutput=False)
    outd = nc.declare_dram_parameter("out", [rows_per_core, out_dim], f16, isOutput=True)

    def ap_custom(tile, base, dims):
        a = tile[:]
        aplist = [list(a.ap[0])] + [[s, n] for (s, n) in dims]
        return AP(a.tensor, a.offset + base, aplist)

    cvals = plan['cvals']
    with TileContext(nc) as tc:
        with (
            tc.tile_pool(name="const", bufs=1) as cstp,
            tc.tile_pool(name="io", bufs=4) as iop,
            tc.tile_pool(name="pp", bufs=4) as ppp,
            tc.tile_pool(name="ps", bufs=2, space="PSUM") as psp,
        ):
            # one-time: diag(c) stationary tiles built from an identity
            IDT = cstp.tile([P_DIM, P_DIM], f16, tag="IDT")
            nc.gpsimd.memset(IDT[:], 1.0)
            nc.gpsimd.affine_select(
                IDT[:], IDT[:], pattern=[[1, P_DIM]],
                compare_op=mybir.AluOpType.is_equal, fill=0.0,
                base=0, channel_multiplier=-1)
            DIAG = {}
            for c in cvals:
                D = cstp.tile([P_DIM, P_DIM], f16, tag=f"D{len(DIAG)}")
                nc.scalar.mul(D[:], IDT[:], float(c))
                DIAG[c] = D
            ZT = cstp.tile([P_DIM, BANK_F32], f16, tag="ZT")
            nc.gpsimd.memset(ZT[:], 0.0)
            CZ = cstp.tile([P_DIM, 2], f16, tag="CZ")
            nc.vector.memset(CZ[:], 0.0)

            def dram_group_ap(dram, g, width):
                # [128p, T, width] view of rows [g*T*128, (g+1)*T*128)
                a = dram[:]
                return AP(a.tensor, g * T * P_DIM * width,
                          [[width, P_DIM], [P_DIM * width, T], [1, width]])

            csz = plan['psize']
            pool_qs = [q for q in plan['sb_instrs'] if q['eng'] == 'pool']
            P_hist = []
            for it in range(n_groups * repeat):
                g = it % n_groups
                warm = it >= 1   # a previous iteration exists
                X1 = iop.tile([P_DIM, T * rep_dim], f16, tag="X1")
                X2 = iop.tile([P_DIM, T * rep_dim], f16, tag="X2")
                O = iop.tile([P_DIM, T * out_dim], f16, tag="O")
                PS = psp.tile([P_DIM, plan['n_banks'] * BANK_F32], f32, tag="PS")
                # scratch tiles are single-engine or pairwise so the
                # first toucher of a recycled buffer inherits at most one
                # foreign engine's tile-reuse wait
                SCD = iop.tile([P_DIM, 8], f16, tag="SCD")    # DVE only
                SDA = iop.tile([P_DIM, 2], f16, tag="SDA")    # DVE -> Act
                SPD = iop.tile([P_DIM, 2], f16, tag="SPD")    # Pool -> DVE
                SED = iop.tile([P_DIM, 4], f16, tag="SED")    # DVE -> PE
                SEA = iop.tile([P_DIM, 2], f16, tag="SEA")    # Act -> PE
                P = ppp.tile([P_DIM, T * csz], f16, tag="P")

                # Every ISA struct holds only 1-2 embedded sync waits, and
                # the tile framework expands transitive vector clocks into
                # explicit waits.  Each engine therefore "observes the
                # world" through a chain of absorber instructions, each
                # adding at most ~2 new clock components, so the hot
                # instructions carry at most one wait.
                # Pool marker: observes the DVE products that wrote this
                # P buffer 4 iterations ago, so the input DMAs' WAR waits
                # elide (the DMA then carries only its queue sem).
                # Pool markers write into X1/X2 at the bytes the last
                # product reads: they are the first toucher of the recycled
                # input tiles, absorbing the DVE write-after-read waits;
                # nosync deps keep the DMA prefetch pass behind them.
                markers = []
                if it >= 4:
                    markers.append(nc.gpsimd.tensor_copy(
                        X1[:, plan['xl1']:plan['xl1'] + 2], CZ[:, 0:2]))
                    markers.append(nc.gpsimd.tensor_copy(
                        X2[:, plan['xl2']:plan['xl2'] + 2], CZ[:, 0:2]))
                if it >= 4:
                    nc.gpsimd.tensor_copy(
                        SPD[:, 0:2], P_hist[-4][:, T * csz - 2:T * csz])
                d1 = nc.gpsimd.dma_start(X1[:], dram_group_ap(x1d, g, rep_dim))
                d2 = nc.gpsimd.dma_start(X2[:], dram_group_ap(x2d, g, rep_dim))
                if markers:
                    from concourse.bass import InstructionNameOrderedSet
                    dep = InstructionNameOrderedSet()
                    for m in markers:
                        dep.add(m.ins.name)
                    d1.ins.add_nosync_dependencies_from(dep)
                    d2.ins.add_nosync_dependencies_from(dep)
                # DVE absorbers: DMA completion sems, Pool's marker, and the
                # PE ticks of 2 iterations ago (via a PSUM read of the last
                # bank-stop range) so the products carry no waits
                nc.vector.tensor_copy(SDA[:, 0:2], X1[:, 0:2])
                nc.vector.tensor_copy(SCD[:, 0:2], X2[:, 0:2])
                if it >= 4:
                    nc.vector.tensor_copy(SCD[:, 2:4], SPD[:, 0:2])
                if it >= 4:
                    bl = plan['pe_instrs'][-1]
                    ps_off = (bl['bank'] * BANK_F32 + (T - 1) * BANK_SLOTS
                              + bl['slot'] + bl['w'] - 2)
                    nc.vector.tensor_copy(SED[:, 0:2],
                                          ap_custom(PS, ps_off, [(1, 2)]))
                nc.vector.tensor_copy(SED[:, 2:4], SDA[:, 0:2])
                # Act observes its own PE-pair tile, then DVE: two
                # single-wait instructions
                if it >= 4:
                    nc.scalar.copy(SEA[:, 0:2], SEA[:, 0:2])
                nc.scalar.copy(SEA[:, 0:2], SDA[:, 0:2])

                for pi in plan['prod_instrs']:
                    dims = [(csz, T), (pi['ds'], pi['n']), (1, pi['d'])]
                    eng = nc.vector if pi['eng'] == 'dve' else nc.gpsimd
                    eng.tensor_tensor(
                        ap_custom(P, pi['pslot'], dims),
                        ap_custom(X1, pi['a'],
                                  [(rep_dim, T), (pi['da'], pi['n']), (1, pi['d'])]),
                        ap_custom(X2, pi['b'],
                                  [(rep_dim, T), (pi['db'], pi['n']), (1, pi['d'])]),
                        mybir.AluOpType.mult,
                    )

                # PE absorbers: garbage ldweights reading the observer
                # scratch; each pulls in the producer's clock so the real
                # matmuls carry at most one wait.  The first real ldweights
                # overwrites the PE array anyway.
                nc.tensor.ldweights(SED[:, 2:4])
                if it >= 4:
                    nc.tensor.ldweights(SED[:, 0:2])
                nc.tensor.ldweights(SEA[:, 0:2])

                # PE scatter-add: diag(c) matmuls accumulating in PSUM.
                # One matmul per row-tile t: a single matmul's PSUM out AP
                # must be flat and stay within one 2KB bank.
                c0 = cvals[0]
                first_in_bank = {}
                for qi in plan['pe_instrs']:
                    first_in_bank.setdefault(qi['bank'], qi)
                for b in range(plan['n_banks']):
                    fq = first_in_bank[b]
                    out_ap = ap_custom(PS, b * BANK_F32 + fq['slot'],
                                       [(1, fq['w'])])
                    nc.tensor.matmul(out_ap, DIAG[c0][:], ZT[:, 0:fq['w']],
                                     start=True, stop=False)
                for qi in plan['pe_instrs']:
                    for t in range(T):
                        out_ap = ap_custom(
                            PS, qi['bank'] * BANK_F32 + t * BANK_SLOTS + qi['slot'],
                            [(1, qi['w'])])
                        mov_ap = ap_custom(P, t * csz + qi['pslot'],
                                           [(1, qi['w'])])
                        nc.tensor.matmul(out_ap, DIAG[qi['c']][:], mov_ap,
                                         start=False,
                                         stop=qi['stop'] and t == T - 1)

                # DVE/Pool: leftover components in SBUF.  Per-engine
                # absorbers: read the LAST product's range (observes the max
                # DVE tick -> later product-RAW waits elide) and touch O to
                # absorb the WAR against the out-DMA two groups back.
                nc.vector.tensor_copy(SCD[:, 6:8],
                                      P[:, T * csz - 2:T * csz])
                if pool_qs:
                    # Pool observes the MAX product tick any of its accums
                    # needs (Q7 gets no transitive elision, so read the
                    # last product's range): STTs then carry only their
                    # own-engine wait
                    nc.gpsimd.tensor_copy(SPD[:, 0:2],
                                          P[:, T * csz - 2:T * csz])
                    for k, qc in enumerate({q['pslot']: q
                                            for q in pool_qs}.values()):
                        if k >= 3:
                            break
                        nc.gpsimd.tensor_copy(
                            SPD[:, 0:2],
                            P[:, qc['pslot'] * T:qc['pslot'] * T + 2])
                if it >= 4:
                    seen_comp = set()
                    for qi in plan['sb_instrs']:
                        cmk = (qi['eng'], qi['o'] // 32)
                        if cmk in seen_comp:
                            continue
                        seen_comp.add(cmk)
                        eng = nc.vector if qi['eng'] == 'dve' else nc.gpsimd
                        eng.tensor_copy(ap_custom(O, qi['o'], [(1, 2)]),
                                        SCD[:, 6:8])
                for qi in plan['sb_instrs']:
                    w = qi['n'] * qi['d']
                    o_ap = ap_custom(O, qi['o'], [(out_dim, T), (1, w)])
                    p_ap = ap_custom(P, qi['pslot'], [(csz, T), (1, w)])
                    eng = nc.vector if qi['eng'] == 'dve' else nc.gpsimd
                    if qi['kind'] == 'TS':
                        eng.tensor_scalar_mul(o_ap, p_ap, float(qi['c']))
                    else:
                        eng.scalar_tensor_tensor(
                            out=o_ap, in0=p_ap, scalar=float(qi['c']),
                            in1=o_ap,
                            op0=mybir.AluOpType.mult,
                            op1=mybir.AluOpType.add,
                        )

                # Act: absorb the out-DMA(g-2) WAR once, then evict
                # PSUM -> fp16 output tile (single PE wait each)
                if it >= 4:
                    for ev in plan['evicts']:
                        nc.scalar.copy(ap_custom(O, ev['o'], [(1, 2)]),
                                       SDA[:, 0:2])
                for ev in plan['evicts']:
                    src = ap_custom(PS, ev['bank'] * BANK_F32 + ev['slot'],
                                    [(BANK_SLOTS, T), (1, ev['w'])])
                    dst = ap_custom(O, ev['o'], [(out_dim, T), (1, ev['w'])])
                    nc.scalar.copy(dst, src)
                # Act gates the non-Act writers of O (DVE accums) with an
                # in-place 2-element copy carrying that single wait, then
                # issues the out-DMA itself: all O writers are then its own
                # in-order engine, leaving only the DMA queue wait.
                dve_qs = [q for q in plan['sb_instrs'] if q['eng'] == 'dve']
                if dve_qs:
                    ql = dve_qs[-1]
                    ap = ap_custom(O, ql['o'] + ql['n'] * ql['d'] - 2, [(1, 2)])
                    nc.scalar.copy(ap, ap)
                if pool_qs:
                    ql = pool_qs[-1]
                    ap = ap_custom(O, ql['o'] + ql['n'] * ql['d'] - 2, [(1, 2)])
                    nc.scalar.copy(ap, ap)
                nc.scalar.dma_start(dram_group_ap(outd, g, out_dim), O[:])
                P_hist.append(P)
    return nc


# ----------------------------------------------------------------------------
# Entry point
# ----------------------------------------------------------------------------

def kernel(x1, x2, cg_tilde, repids_in1, repids_in2, repids_out, out_dim):
    from concourse.bass_utils import run_bass_kernel_spmd

    x1 = np.asarray(x1, dtype=np.float16)
    x2 = np.asarray(x2, dtype=np.float16)
    cg = np.asarray(cg_tilde, dtype=np.float32)
    r1 = np.asarray(repids_in1).astype(np.int64)
    r2 = np.asarray(repids_in2).astype(np.int64)
    ro = np.asarray(repids_out).astype(np.int64)
    out_dim = int(out_dim)

    n, rep_dim = x1.shape
    rows_per_core = n // N_CORES

    key = (rows_per_core, rep_dim, out_dim, cg.tobytes(), r1.tobytes(),
           r2.tobytes(), ro.tobytes())
    cache_key = hash(key)
    if cache_key not in _BUILD_CACHE:
        plan = _build_plan(cg, r1, r2, ro, out_dim)
        nc = _build_bass(plan, rows_per_core, rep_dim, out_dim)
        _BUILD_CACHE[cache_key] = nc
    nc = _BUILD_CACHE[cache_key]

    in_maps = [
        {"x1": x1[i*rows_per_core:(i+1)*rows_per_core],
         "x2": x2[i*rows_per_core:(i+1)*rows_per_core]}
        for i in range(N_CORES)
    ]
    res = run_bass_kernel_spmd(nc, in_maps, list(range(N_CORES)))
    out = np.concatenate([res.results[i]["out"] for i in range(N_CORES)], axis=0)
    return out.astype(np.float32)
